# revision 16
# baseline (speedup 1.0000x reference)
"""Trainium2 Bass kernel for nn_BlockUngrouper.

Problem: out[b, n, :] = block_features[b, g, k, :] where g is the block whose
one-hot claims token n and k is n's rank within that block (cumsum of the
one-hot along n).  The input distribution (per-sample permutation partition)
guarantees each token is claimed by exactly one block and ranks < 128, so per
batch this is a row-permutation gather with
    flat_idx[n] = 128 * g(n) + rank(n).

Sharding: data-parallel over the batch dim, 2 batches per NeuronCore x 8.

Per-core program (all index arithmetic exact in fp32/bf16):
  1. onehot [N, 128] -> SBUF bf16, layout [token-in-tile, (tile, g)]
     (dtype cast during SWDGE DMA).
  2. counts[g, t]: per 128-token tile, PE matmul lhsT=OH rhs=ones.
  3. prefix[g, t]: exclusive scan over tiles (DVE tensor_tensor_scan),
     PE-transpose to [t, g], add 128*g - 1.
  4. per 4-tile PSUM group: flatten the 4 prefix rows to partition 0 (tiny
     SBUF->SBUF DMA), broadcast-add via K=1 matmul (start), then 4 upper-
     triangular-matrix matmuls add the within-tile inclusive cumsum.
  5. flat_idx: DVE scalar_tensor_tensor (PSUM x onehot) with accum_out
     reducing over g (the one-hot selects its block's entry).
  6. per tile: one SWDGE indirect DMA gathers 128 x 1KB feature rows
     (hardware contract: one index per partition); per 8-tile chunk one
     HWDGE store DMA writes the rows to the output.
"""

from contextlib import ExitStack

import numpy as np

import concourse.bass as bass
import concourse.bacc as bacc
import concourse.mybir as mybir
import concourse.tile as tile
from concourse import bass_utils
from concourse.masks import make_identity, make_upper_triangular

P = 128  # partitions = tokens per tile = G (blocks) = NG_MAX
KERNEL_VERSION = 10  # bump on every meaningful kernel change (NEFF-cache buster)
N_CORES = 8
B_FULL = 16  # full batch dim
N_TOK = 16384  # tokens per batch
D_FEAT = 256  # feature dim
NB = B_FULL // N_CORES  # batches per core

FP32 = mybir.dt.float32
BF16 = mybir.dt.bfloat16
I32 = mybir.dt.int32


def build_nc(NB: int, N: int, D: int, CT: int = 8, GRP: int = 4, LC: int = 16,
             STG_BUFS: int = 8, IDX_BUFS: int = 6, REPS: int = 1,
             DYN_LOOP: int = 0, MODE: str = "full", SCRATCH: int = 65536,
             MG: int = 0, Q: int = 1, SALT: int = 0):
    """Build the per-core bass program.

    NB: batches per core; N: tokens per batch; D: feature dim.
    CT: tiles per store chunk; GRP: tiles per PSUM group; LC: tiles per
    onehot load chunk.
    """
    T = N // P  # token tiles per batch
    assert T * P == N
    GRP = min(GRP, T)
    CT = min(CT, T)
    LC = min(LC, T)
    assert T % GRP == 0 and T % CT == 0 and T % LC == 0
    add = mybir.AluOpType.add
    mult = mybir.AluOpType.mult
    bypass = mybir.AluOpType.bypass

    nc = bacc.Bacc("TRN2", target_bir_lowering=False, debug=False,
                   dynamic_dma_scratch_size=SCRATCH, num_swdge_queues=Q)

    def _q(inst, i):
        qi = i % Q
        if qi:
            inst.ins.queue = f"qPoolDynamic{qi}"
        return inst

    def _store_eng(i):
        if SALT and (i % 2):
            return nc.scalar
        return nc.sync

    feat = nc.dram_tensor("block_features", [NB * N, D], FP32, kind="ExternalInput")
    oh = nc.dram_tensor("block_onehot", [NB, N, P], FP32, kind="ExternalInput")
    out = nc.dram_tensor("out", [NB, N, D], FP32, kind="ExternalOutput")
    gidx = None
    if MODE == "gather_in":
        # debug: externally supplied flat indices, [p, t] layout per batch
        gidx = nc.dram_tensor("gidx", [NB, P, N // P], I32, kind="ExternalInput")
    # The PJRT NEFF cache keys on the HLO alone (the embedded bass program
    # does not enter the hash), so distinct kernel versions collide.  A dummy
    # input whose shape encodes a version nonce forces a distinct hash.
    import zlib as _zlib
    _nonce = (
        _zlib.crc32(
            f"v{KERNEL_VERSION}-{NB}-{N}-{D}-{CT}-{GRP}-{LC}-{STG_BUFS}-{REPS}-{DYN_LOOP}-{MODE}-{SCRATCH}-{MG}-{Q}-{SALT}".encode()
        )
        % 4093
        + 1
    )
    nc.dram_tensor("version_tag", [1, _nonce], FP32, kind="ExternalInput")

    with tile.TileContext(nc) as tc, ExitStack() as ctx:
        cpool = ctx.enter_context(tc.tile_pool(name="const", bufs=1))
        ohpool = ctx.enter_context(tc.tile_pool(name="ohp", bufs=2))
        ldpool = ctx.enter_context(tc.tile_pool(name="ld", bufs=2))
        wpool = ctx.enter_context(tc.tile_pool(name="work", bufs=2))
        ppool = ctx.enter_context(tc.tile_pool(name="psum", bufs=2, space="PSUM"))
        pspool = ctx.enter_context(tc.tile_pool(name="psumsm", bufs=2, space="PSUM"))
        spool = ctx.enter_context(tc.tile_pool(name="stage", bufs=STG_BUFS))
        fpool = ctx.enter_context(tc.tile_pool(name="flat", bufs=2))
        ipool = ctx.enter_context(tc.tile_pool(name="idx", bufs=IDX_BUFS))

        # --- constants ---
        triu = cpool.tile([P, P], BF16)  # triu[k, m] = 1 iff k <= m
        make_upper_triangular(nc, triu[:], val=1.0, diag=True)
        ident = cpool.tile([P, P], FP32)
        make_identity(nc, ident[:])
        ones_col = cpool.tile([P, 1], BF16)
        nc.gpsimd.memset(ones_col[:], 1.0)
        ones_row = cpool.tile([1, P], FP32)
        nc.gpsimd.memset(ones_row[:], 1.0)
        # gmat[t, g] = 128*g - 1 (same every row)
        gmat_i = cpool.tile([P, P], I32)
        nc.gpsimd.iota(gmat_i[:], pattern=[[P, P]], base=-1, channel_multiplier=0)
        gmat = cpool.tile([P, P], FP32)
        nc.vector.tensor_copy(gmat[:], gmat_i[:])

        import contextlib
        loop_cm = tc.For_i(0, DYN_LOOP, 1) if DYN_LOOP else contextlib.nullcontext()
        with loop_cm:
          for rep in range(REPS):
            for b in range(NB):
                oh_src = oh.ap()[b].rearrange("(t p) g -> p t g", p=P)
                out_dst = out.ap()[b].rearrange("(t p) d -> p t d", p=P)

                if MODE == "gather_in":
                    idxs = ldpool.tile([P, T], I32, tag="gidx")
                    nc.scalar.dma_start(out=idxs[:], in_=gidx.ap()[b])
                    for c in range(T // CT):
                        stg = spool.tile([P, CT * D], FP32, tag="stg")
                        if MG:
                            idx_c = ipool.tile([P, CT], I32, tag="idxc")
                            nc.vector.tensor_copy(
                                idx_c[:], idxs[:, c * CT : (c + 1) * CT]
                            )
                            nc.gpsimd.indirect_dma_start(
                                out=stg[:],
                                out_offset=None,
                                in_=feat.ap(),
                                in_offset=bass.IndirectOffsetOnAxis(
                                    ap=idx_c[:], axis=0
                                ),
                                element_offset=b * N * D,
                            )
                        else:
                            for i in range(CT):
                                t = c * CT + i
                                nc.gpsimd.indirect_dma_start(
                                    out=stg[:, i * D : (i + 1) * D],
                                    out_offset=None,
                                    in_=feat.ap(),
                                    in_offset=bass.IndirectOffsetOnAxis(
                                        ap=idxs[:, t : t + 1], axis=0
                                    ),
                                    element_offset=b * N * D,
                                )
                        nc.sync.dma_start(
                            out=out_dst[:, c * CT : (c + 1) * CT, :], in_=stg[:]
                        )
                    continue
                if MODE == "gather":
                    idx_mat = wpool.tile([P, P], I32, tag="idxmat")
                    nc.gpsimd.iota(idx_mat[:, :T], pattern=[[1, T]], base=0,
                                   channel_multiplier=T)
                    for c in range(T // CT):
                        stg = spool.tile([P, CT * D], FP32, tag="stg")
                        if MG:
                            nc.gpsimd.indirect_dma_start(
                                out=stg[:],
                                out_offset=None,
                                in_=feat.ap(),
                                in_offset=bass.IndirectOffsetOnAxis(
                                    ap=idx_mat[:, c * CT : (c + 1) * CT], axis=0
                                ),
                                element_offset=b * N * D,
                            )
                        else:
                            for i in range(CT):
                                t = c * CT + i
                                nc.gpsimd.indirect_dma_start(
                                    out=stg[:, i * D : (i + 1) * D],
                                    out_offset=None,
                                    in_=feat.ap(),
                                    in_offset=bass.IndirectOffsetOnAxis(
                                        ap=idx_mat[:, t : t + 1], axis=0
                                    ),
                                    element_offset=b * N * D,
                                )
                        nc.sync.dma_start(
                            out=out_dst[:, c * CT : (c + 1) * CT, :], in_=stg[:]
                        )
                    continue
                oh_sb = ohpool.tile([P, T * P], BF16, tag="oh")
                countsT_ps = pspool.tile([P, T], FP32, tag="counts")
                incl = wpool.tile([P, P], FP32, tag="incl")  # [g, t] inclusive
                pexT = wpool.tile([P, P], FP32, tag="pexT")  # [g, t] exclusive
                flat_mat = wpool.tile([P, P], FP32, tag="flatmat")  # [p', t]
                idx_mat = wpool.tile([P, P], I32, tag="idxmat")
                PCH = min(16, T)  # tiles per prefix/transpose slice
                for s in range(T // PCH):
                    s0 = s * PCH
                    # --- load + cast + counts + scan per LC chunk ---
                    for lc in range(s0 // LC, (s0 + PCH) // LC):
                        lc0, lc1 = lc * LC, (lc + 1) * LC
                        ld = ldpool.tile([P, LC * P], FP32, tag="ld")
                        nc.scalar.dma_start(
                            out=ld[:], in_=oh_src[:, lc0:lc1, :]
                        )
                        nc.vector.tensor_copy(
                            oh_sb[:, lc0 * P : lc1 * P], ld[:]
                        )
                        for t in range(lc0, lc1):
                            nc.tensor.matmul(
                                out=countsT_ps[:, t : t + 1],
                                lhsT=oh_sb[:, t * P : (t + 1) * P],
                                rhs=ones_col[:],
                                start=True,
                                stop=True,
                            )
                        nc.vector.tensor_tensor_scan(
                            out=incl[:, lc0:lc1],
                            data0=countsT_ps[:, lc0:lc1],
                            data1=gmat[:, 0:LC],
                            initial=(0.0 if lc == 0 else incl[:, lc0 - 1 : lc0]),
                            op0=add,
                            op1=bypass,
                        )
                        nc.vector.tensor_tensor(
                            out=pexT[:, lc0:lc1],
                            in0=incl[:, lc0:lc1],
                            in1=countsT_ps[:, lc0:lc1],
                            op=mybir.AluOpType.subtract,
                        )
                    # --- transpose slice into partition-0 tiles + gmat add ---
                    pref_ps = pspool.tile([PCH, P], FP32, tag="preft")
                    nc.tensor.transpose(
                        out=pref_ps[:],
                        in_=pexT[:, s0 : s0 + PCH],
                        identity=ident[:],
                    )
                    pref_adj = wpool.tile([PCH, P], FP32, tag="prefadj")
                    nc.vector.tensor_tensor(
                        out=pref_adj[:],
                        in0=pref_ps[:],
                        in1=gmat[0:PCH, :],
                        op=add,
                    )
                    # --- groups: prefix broadcast + within-tile cumsum + select ---
                    for grp in range(s0 // GRP, (s0 + PCH) // GRP):
                        flat_row = fpool.tile([1, GRP * P], FP32, tag="flatrow")
                        g_in_s = grp - s0 // GRP
                        nc.scalar.dma_start(
                            out=flat_row[:],
                            in_=pref_adj[g_in_s * GRP : (g_in_s + 1) * GRP, :],
                        )
                        grp_ps = ppool.tile([P, GRP * P], FP32, tag="grp")
                        nc.tensor.matmul(
                            out=grp_ps[:],
                            lhsT=ones_row[:],
                            rhs=flat_row[:],
                            start=True,
                            stop=False,
                            skip_group_check=True,
                        )
                        for i in range(GRP):
                            t = grp * GRP + i
                            nc.tensor.matmul(
                                out=grp_ps[:, i * P : (i + 1) * P],
                                lhsT=triu[:],
                                rhs=oh_sb[:, t * P : (t + 1) * P],
                                start=False,
                                stop=True,
                                skip_group_check=True,
                            )
                        scratch = wpool.tile([P, GRP * P], FP32, tag="scratch")
                        for i in range(GRP):
                            t = grp * GRP + i
                            nc.vector.scalar_tensor_tensor(
                                out=scratch[:, i * P : (i + 1) * P],
                                in0=grp_ps[:, i * P : (i + 1) * P],
                                scalar=1.0,
                                in1=oh_sb[:, t * P : (t + 1) * P],
                                op0=mult,
                                op1=mult,
                                accum_out=flat_mat[:, t : t + 1],
                            )
                    if MODE == "index":
                        nc.sync.dma_start(
                            out=out_dst[:, s0 : s0 + 1, 0:1],
                            in_=flat_mat[:, s0 : s0 + 1],
                        )
                        continue
                    # --- gathers + stores for this slice ---
                    for c in range(s0 // CT, (s0 + PCH) // CT):
                        stg = spool.tile([P, CT * D], FP32, tag="stg")
                        if MG:
                            idx_c = ipool.tile([P, CT], I32, tag="idxc")
                            nc.vector.tensor_copy(
                                idx_c[:], flat_mat[:, c * CT : (c + 1) * CT]
                            )
                            nc.gpsimd.indirect_dma_start(
                                out=stg[:],
                                out_offset=None,
                                in_=feat.ap(),
                                in_offset=bass.IndirectOffsetOnAxis(
                                    ap=idx_c[:], axis=0
                                ),
                                element_offset=b * N * D,
                            )
                        else:
                            nc.vector.tensor_copy(
                                idx_mat[:, c * CT : (c + 1) * CT],
                                flat_mat[:, c * CT : (c + 1) * CT],
                            )
                            for i in range(CT):
                                t = c * CT + i
                                nc.gpsimd.indirect_dma_start(
                                    out=stg[:, i * D : (i + 1) * D],
                                    out_offset=None,
                                    in_=feat.ap(),
                                    in_offset=bass.IndirectOffsetOnAxis(
                                        ap=idx_mat[:, t : t + 1], axis=0
                                    ),
                                    element_offset=b * N * D,
                                )
                        nc.sync.dma_start(
                            out=out_dst[:, c * CT : (c + 1) * CT, :], in_=stg[:]
                        )

    nc.compile()
    return nc


_NC_CACHE = {}


def _get_nc():
    key = (NB, N_TOK, D_FEAT)
    if key not in _NC_CACHE:
        _NC_CACHE[key] = build_nc(*key)
    return _NC_CACHE[key]


def _tag_shape(nc):
    for alloc in nc.m.functions[0].allocations:
        if isinstance(alloc, mybir.MemoryLocationSet) and alloc.kind == "ExternalInput":
            if alloc.memorylocations[0].name == "version_tag":
                return tuple(alloc.tensor_shape)
    return None


def fix_maps(nc, in_maps):
    """Adjust the version_tag entry of in_maps to match nc's nonce shape."""
    shape = _tag_shape(nc)
    maps = [dict(m) for m in in_maps]
    for m in maps:
        m.pop("version_tag", None)
        if shape is not None:
            m["version_tag"] = np.zeros(shape, np.float32)
    return maps


def make_in_maps(block_features: np.ndarray, block_onehot: np.ndarray):
    """Shard full inputs batch-wise into 8 per-core input maps."""
    feat = np.ascontiguousarray(block_features, dtype=np.float32).reshape(
        B_FULL, N_TOK, D_FEAT
    )
    oh = np.ascontiguousarray(block_onehot, dtype=np.float32)
    nc = _get_nc()
    tag_shape = _tag_shape(nc)
    in_maps = []
    for c in range(N_CORES):
        lo, hi = c * NB, (c + 1) * NB
        m = {
            "block_features": feat[lo:hi].reshape(NB * N_TOK, D_FEAT),
            "block_onehot": oh[lo:hi],
        }
        if tag_shape is not None:
            m["version_tag"] = np.zeros(tag_shape, np.float32)
        in_maps.append(m)
    return in_maps


def run_spmd(in_maps, trace: bool = False):
    """Compile (cached) + run the SPMD program on cores 0-7."""
    nc = _get_nc()
    return bass_utils.run_bass_kernel_spmd(
        nc, in_maps, core_ids=list(range(N_CORES)), trace=trace
    )


def kernel(**inputs) -> np.ndarray:
    block_features = inputs["block_features"]
    block_onehot = inputs["block_onehot"]
    in_maps = make_in_maps(block_features, block_onehot)
    res = run_spmd(in_maps, trace=False)
    out = np.concatenate([r["out"] for r in res.results], axis=0)
    return out.reshape(B_FULL, N_TOK, D_FEAT)



# revision 18
# speedup vs baseline: 1.0743x; 1.0743x over previous
"""Trainium2 Bass kernel for nn_BlockUngrouper.

Problem: out[b, n, :] = block_features[b, g, k, :] where g is the block whose
one-hot claims token n and k is n's rank within that block (cumsum of the
one-hot along n).  The input distribution (per-sample permutation partition)
guarantees each token is claimed by exactly one block and ranks < 128, so per
batch this is a row-permutation gather with
    flat_idx[n] = 128 * g(n) + rank(n).

Sharding: data-parallel over the batch dim, 2 batches per NeuronCore x 8.

Per-core program (all index arithmetic exact in fp32/bf16):
  1. onehot [N, 128] -> SBUF bf16, layout [token-in-tile, (tile, g)]
     (dtype cast during SWDGE DMA).
  2. counts[g, t]: per 128-token tile, PE matmul lhsT=OH rhs=ones.
  3. prefix[g, t]: exclusive scan over tiles (DVE tensor_tensor_scan),
     PE-transpose to [t, g], add 128*g - 1.
  4. per 4-tile PSUM group: flatten the 4 prefix rows to partition 0 (tiny
     SBUF->SBUF DMA), broadcast-add via K=1 matmul (start), then 4 upper-
     triangular-matrix matmuls add the within-tile inclusive cumsum.
  5. flat_idx: DVE scalar_tensor_tensor (PSUM x onehot) with accum_out
     reducing over g (the one-hot selects its block's entry).
  6. per tile: one SWDGE indirect DMA gathers 128 x 1KB feature rows
     (hardware contract: one index per partition); per 8-tile chunk one
     HWDGE store DMA writes the rows to the output.
"""

from contextlib import ExitStack

import numpy as np

import concourse.bass as bass
import concourse.bacc as bacc
import concourse.mybir as mybir
import concourse.tile as tile
from concourse import bass_utils
from concourse.masks import make_identity, make_upper_triangular

P = 128  # partitions = tokens per tile = G (blocks) = NG_MAX
KERNEL_VERSION = 10  # bump on every meaningful kernel change (NEFF-cache buster)
N_CORES = 8
B_FULL = 16  # full batch dim
N_TOK = 16384  # tokens per batch
D_FEAT = 256  # feature dim
NB = B_FULL // N_CORES  # batches per core

FP32 = mybir.dt.float32
BF16 = mybir.dt.bfloat16
I32 = mybir.dt.int32


def build_nc(NB: int, N: int, D: int, CT: int = 8, GRP: int = 4, LC: int = 16,
             STG_BUFS: int = 8, IDX_BUFS: int = 6, REPS: int = 1,
             DYN_LOOP: int = 0, MODE: str = "full", SCRATCH: int = 65536,
             MG: int = 0, Q: int = 1, SALT: int = 0):
    """Build the per-core bass program.

    NB: batches per core; N: tokens per batch; D: feature dim.
    CT: tiles per store chunk; GRP: tiles per PSUM group; LC: tiles per
    onehot load chunk.
    """
    T = N // P  # token tiles per batch
    assert T * P == N
    GRP = min(GRP, T)
    CT = min(CT, T)
    LC = min(LC, T)
    assert T % GRP == 0 and T % CT == 0 and T % LC == 0
    add = mybir.AluOpType.add
    mult = mybir.AluOpType.mult
    bypass = mybir.AluOpType.bypass

    nc = bacc.Bacc("TRN2", target_bir_lowering=False, debug=False,
                   dynamic_dma_scratch_size=SCRATCH, num_swdge_queues=Q)

    def _q(inst, i):
        qi = i % Q
        if qi:
            inst.ins.queue = f"qPoolDynamic{qi}"
        return inst

    def _store_eng(i):
        if SALT and (i % 2):
            return nc.scalar
        return nc.sync

    feat = nc.dram_tensor("block_features", [NB * N, D], FP32, kind="ExternalInput")
    oh = nc.dram_tensor("block_onehot", [NB, N, P], FP32, kind="ExternalInput")
    out = nc.dram_tensor("out", [NB, N, D], FP32, kind="ExternalOutput")
    gidx = None
    if MODE == "gather_in":
        # debug: externally supplied flat indices, [p, t] layout per batch
        gidx = nc.dram_tensor("gidx", [NB, P, N // P], I32, kind="ExternalInput")
    # The PJRT NEFF cache keys on the HLO alone (the embedded bass program
    # does not enter the hash), so distinct kernel versions collide.  A dummy
    # input whose shape encodes a version nonce forces a distinct hash.
    import zlib as _zlib
    _nonce = (
        _zlib.crc32(
            f"v{KERNEL_VERSION}-{NB}-{N}-{D}-{CT}-{GRP}-{LC}-{STG_BUFS}-{REPS}-{DYN_LOOP}-{MODE}-{SCRATCH}-{MG}-{Q}-{SALT}".encode()
        )
        % 4093
        + 1
    )
    nc.dram_tensor("version_tag", [1, _nonce], FP32, kind="ExternalInput")

    with tile.TileContext(nc) as tc, ExitStack() as ctx:
        cpool = ctx.enter_context(tc.tile_pool(name="const", bufs=1))
        ohpool = ctx.enter_context(tc.tile_pool(name="ohp", bufs=2))
        ldpool = ctx.enter_context(tc.tile_pool(name="ld", bufs=2))
        wpool = ctx.enter_context(tc.tile_pool(name="work", bufs=2))
        ppool = ctx.enter_context(tc.tile_pool(name="psum", bufs=2, space="PSUM"))
        pspool = ctx.enter_context(tc.tile_pool(name="psumsm", bufs=2, space="PSUM"))
        spool = ctx.enter_context(tc.tile_pool(name="stage", bufs=STG_BUFS))
        fpool = ctx.enter_context(tc.tile_pool(name="flat", bufs=2))
        ipool = ctx.enter_context(tc.tile_pool(name="idx", bufs=IDX_BUFS))

        # --- constants ---
        triu = cpool.tile([P, P], BF16)  # triu[k, m] = 1 iff k <= m
        make_upper_triangular(nc, triu[:], val=1.0, diag=True)
        ident = cpool.tile([P, P], FP32)
        make_identity(nc, ident[:])
        ones_col = cpool.tile([P, 1], BF16)
        nc.gpsimd.memset(ones_col[:], 1.0)
        ones_row = cpool.tile([1, P], FP32)
        nc.gpsimd.memset(ones_row[:], 1.0)
        # gmat[t, g] = 128*g - 1 (same every row)
        gmat_i = cpool.tile([P, P], I32)
        nc.gpsimd.iota(gmat_i[:], pattern=[[P, P]], base=-1, channel_multiplier=0)
        gmat = cpool.tile([P, P], FP32)
        nc.vector.tensor_copy(gmat[:], gmat_i[:])

        import contextlib
        loop_cm = tc.For_i(0, DYN_LOOP, 1) if DYN_LOOP else contextlib.nullcontext()
        with loop_cm:
          for rep in range(REPS):
            for b in range(NB):
                oh_src = oh.ap()[b].rearrange("(t p) g -> p t g", p=P)
                out_dst = out.ap()[b].rearrange("(t p) d -> p t d", p=P)

                if MODE == "gather_in":
                    idxs = ldpool.tile([P, T], I32, tag="gidx")
                    nc.scalar.dma_start(out=idxs[:], in_=gidx.ap()[b])
                    for c in range(T // CT):
                        stg = spool.tile([P, CT * D], FP32, tag="stg")
                        if MG:
                            idx_c = ipool.tile([P, CT], I32, tag="idxc")
                            nc.vector.tensor_copy(
                                idx_c[:], idxs[:, c * CT : (c + 1) * CT]
                            )
                            nc.gpsimd.indirect_dma_start(
                                out=stg[:],
                                out_offset=None,
                                in_=feat.ap(),
                                in_offset=bass.IndirectOffsetOnAxis(
                                    ap=idx_c[:], axis=0
                                ),
                                element_offset=b * N * D,
                            )
                        else:
                            for i in range(CT):
                                t = c * CT + i
                                _q(nc.gpsimd.indirect_dma_start(
                                    out=stg[:, i * D : (i + 1) * D],
                                    out_offset=None,
                                    in_=feat.ap(),
                                    in_offset=bass.IndirectOffsetOnAxis(
                                        ap=idxs[:, t : t + 1], axis=0
                                    ),
                                    element_offset=b * N * D,
                                ), t)
                        _store_eng(c).dma_start(
                            out=out_dst[:, c * CT : (c + 1) * CT, :], in_=stg[:]
                        )
                    continue
                if MODE == "gather":
                    idx_mat = wpool.tile([P, P], I32, tag="idxmat")
                    nc.gpsimd.iota(idx_mat[:, :T], pattern=[[1, T]], base=0,
                                   channel_multiplier=T)
                    for c in range(T // CT):
                        stg = spool.tile([P, CT * D], FP32, tag="stg")
                        if MG:
                            nc.gpsimd.indirect_dma_start(
                                out=stg[:],
                                out_offset=None,
                                in_=feat.ap(),
                                in_offset=bass.IndirectOffsetOnAxis(
                                    ap=idx_mat[:, c * CT : (c + 1) * CT], axis=0
                                ),
                                element_offset=b * N * D,
                            )
                        else:
                            for i in range(CT):
                                t = c * CT + i
                                _q(nc.gpsimd.indirect_dma_start(
                                    out=stg[:, i * D : (i + 1) * D],
                                    out_offset=None,
                                    in_=feat.ap(),
                                    in_offset=bass.IndirectOffsetOnAxis(
                                        ap=idx_mat[:, t : t + 1], axis=0
                                    ),
                                    element_offset=b * N * D,
                                ), t)
                        _store_eng(c).dma_start(
                            out=out_dst[:, c * CT : (c + 1) * CT, :], in_=stg[:]
                        )
                    continue
                oh_sb = ohpool.tile([P, T * P], BF16, tag="oh")
                countsT_ps = pspool.tile([P, T], FP32, tag="counts")
                incl = wpool.tile([P, P], FP32, tag="incl")  # [g, t] inclusive
                pexT = wpool.tile([P, P], FP32, tag="pexT")  # [g, t] exclusive
                flat_mat = wpool.tile([P, P], FP32, tag="flatmat")  # [p', t]
                idx_mat = wpool.tile([P, P], I32, tag="idxmat")
                PCH = min(16, T)  # tiles per prefix/transpose slice
                for s in range(T // PCH):
                    s0 = s * PCH
                    # --- load + cast + counts + scan per LC chunk ---
                    for lc in range(s0 // LC, (s0 + PCH) // LC):
                        lc0, lc1 = lc * LC, (lc + 1) * LC
                        ld = ldpool.tile([P, LC * P], FP32, tag="ld")
                        nc.scalar.dma_start(
                            out=ld[:], in_=oh_src[:, lc0:lc1, :]
                        )
                        nc.vector.tensor_copy(
                            oh_sb[:, lc0 * P : lc1 * P], ld[:]
                        )
                        for t in range(lc0, lc1):
                            nc.tensor.matmul(
                                out=countsT_ps[:, t : t + 1],
                                lhsT=oh_sb[:, t * P : (t + 1) * P],
                                rhs=ones_col[:],
                                start=True,
                                stop=True,
                            )
                        nc.vector.tensor_tensor_scan(
                            out=incl[:, lc0:lc1],
                            data0=countsT_ps[:, lc0:lc1],
                            data1=gmat[:, 0:LC],
                            initial=(0.0 if lc == 0 else incl[:, lc0 - 1 : lc0]),
                            op0=add,
                            op1=bypass,
                        )
                        nc.vector.tensor_tensor(
                            out=pexT[:, lc0:lc1],
                            in0=incl[:, lc0:lc1],
                            in1=countsT_ps[:, lc0:lc1],
                            op=mybir.AluOpType.subtract,
                        )
                    # --- transpose slice into partition-0 tiles + gmat add ---
                    pref_ps = pspool.tile([PCH, P], FP32, tag="preft")
                    nc.tensor.transpose(
                        out=pref_ps[:],
                        in_=pexT[:, s0 : s0 + PCH],
                        identity=ident[:],
                    )
                    pref_adj = wpool.tile([PCH, P], FP32, tag="prefadj")
                    nc.vector.tensor_tensor(
                        out=pref_adj[:],
                        in0=pref_ps[:],
                        in1=gmat[0:PCH, :],
                        op=add,
                    )
                    # --- groups: prefix broadcast + within-tile cumsum + select ---
                    for grp in range(s0 // GRP, (s0 + PCH) // GRP):
                        flat_row = fpool.tile([1, GRP * P], FP32, tag="flatrow")
                        g_in_s = grp - s0 // GRP
                        nc.scalar.dma_start(
                            out=flat_row[:],
                            in_=pref_adj[g_in_s * GRP : (g_in_s + 1) * GRP, :],
                        )
                        grp_ps = ppool.tile([P, GRP * P], FP32, tag="grp")
                        nc.tensor.matmul(
                            out=grp_ps[:],
                            lhsT=ones_row[:],
                            rhs=flat_row[:],
                            start=True,
                            stop=False,
                            skip_group_check=True,
                        )
                        for i in range(GRP):
                            t = grp * GRP + i
                            nc.tensor.matmul(
                                out=grp_ps[:, i * P : (i + 1) * P],
                                lhsT=triu[:],
                                rhs=oh_sb[:, t * P : (t + 1) * P],
                                start=False,
                                stop=True,
                                skip_group_check=True,
                            )
                        scratch = wpool.tile([P, GRP * P], FP32, tag="scratch")
                        for i in range(GRP):
                            t = grp * GRP + i
                            nc.vector.scalar_tensor_tensor(
                                out=scratch[:, i * P : (i + 1) * P],
                                in0=grp_ps[:, i * P : (i + 1) * P],
                                scalar=1.0,
                                in1=oh_sb[:, t * P : (t + 1) * P],
                                op0=mult,
                                op1=mult,
                                accum_out=flat_mat[:, t : t + 1],
                            )
                    if MODE == "index":
                        nc.sync.dma_start(
                            out=out_dst[:, s0 : s0 + 1, 0:1],
                            in_=flat_mat[:, s0 : s0 + 1],
                        )
                        continue
                    # --- gathers + stores for this slice ---
                    for c in range(s0 // CT, (s0 + PCH) // CT):
                        stg = spool.tile([P, CT * D], FP32, tag="stg")
                        if MG:
                            idx_c = ipool.tile([P, CT], I32, tag="idxc")
                            nc.vector.tensor_copy(
                                idx_c[:], flat_mat[:, c * CT : (c + 1) * CT]
                            )
                            nc.gpsimd.indirect_dma_start(
                                out=stg[:],
                                out_offset=None,
                                in_=feat.ap(),
                                in_offset=bass.IndirectOffsetOnAxis(
                                    ap=idx_c[:], axis=0
                                ),
                                element_offset=b * N * D,
                            )
                        else:
                            nc.vector.tensor_copy(
                                idx_mat[:, c * CT : (c + 1) * CT],
                                flat_mat[:, c * CT : (c + 1) * CT],
                            )
                            for i in range(CT):
                                t = c * CT + i
                                _q(nc.gpsimd.indirect_dma_start(
                                    out=stg[:, i * D : (i + 1) * D],
                                    out_offset=None,
                                    in_=feat.ap(),
                                    in_offset=bass.IndirectOffsetOnAxis(
                                        ap=idx_mat[:, t : t + 1], axis=0
                                    ),
                                    element_offset=b * N * D,
                                ), t)
                        _store_eng(c).dma_start(
                            out=out_dst[:, c * CT : (c + 1) * CT, :], in_=stg[:]
                        )

    nc.compile()
    return nc


_NC_CACHE = {}


def _get_nc():
    key = (NB, N_TOK, D_FEAT)
    if key not in _NC_CACHE:
        _NC_CACHE[key] = build_nc(*key)
    return _NC_CACHE[key]


def _tag_shape(nc):
    for alloc in nc.m.functions[0].allocations:
        if isinstance(alloc, mybir.MemoryLocationSet) and alloc.kind == "ExternalInput":
            if alloc.memorylocations[0].name == "version_tag":
                return tuple(alloc.tensor_shape)
    return None


def fix_maps(nc, in_maps):
    """Adjust the version_tag entry of in_maps to match nc's nonce shape."""
    shape = _tag_shape(nc)
    maps = [dict(m) for m in in_maps]
    for m in maps:
        m.pop("version_tag", None)
        if shape is not None:
            m["version_tag"] = np.zeros(shape, np.float32)
    return maps


def make_in_maps(block_features: np.ndarray, block_onehot: np.ndarray):
    """Shard full inputs batch-wise into 8 per-core input maps."""
    feat = np.ascontiguousarray(block_features, dtype=np.float32).reshape(
        B_FULL, N_TOK, D_FEAT
    )
    oh = np.ascontiguousarray(block_onehot, dtype=np.float32)
    nc = _get_nc()
    tag_shape = _tag_shape(nc)
    in_maps = []
    for c in range(N_CORES):
        lo, hi = c * NB, (c + 1) * NB
        m = {
            "block_features": feat[lo:hi].reshape(NB * N_TOK, D_FEAT),
            "block_onehot": oh[lo:hi],
        }
        if tag_shape is not None:
            m["version_tag"] = np.zeros(tag_shape, np.float32)
        in_maps.append(m)
    return in_maps


def run_spmd(in_maps, trace: bool = False):
    """Compile (cached) + run the SPMD program on cores 0-7."""
    nc = _get_nc()
    return bass_utils.run_bass_kernel_spmd(
        nc, in_maps, core_ids=list(range(N_CORES)), trace=trace
    )


def kernel(**inputs) -> np.ndarray:
    block_features = inputs["block_features"]
    block_onehot = inputs["block_onehot"]
    in_maps = make_in_maps(block_features, block_onehot)
    res = run_spmd(in_maps, trace=False)
    out = np.concatenate([r["out"] for r in res.results], axis=0)
    return out.reshape(B_FULL, N_TOK, D_FEAT)



# revision 20
# speedup vs baseline: 1.1864x; 1.1044x over previous
"""Trainium2 Bass kernel for nn_BlockUngrouper.

Problem: out[b, n, :] = block_features[b, g, k, :] where g is the block whose
one-hot claims token n and k is n's rank within that block (cumsum of the
one-hot along n).  The input distribution (per-sample permutation partition)
guarantees each token is claimed by exactly one block and ranks < 128, so per
batch this is a row-permutation gather with
    flat_idx[n] = 128 * g(n) + rank(n).

Sharding: data-parallel over the batch dim, 2 batches per NeuronCore x 8.

Per-core program (all index arithmetic exact in fp32/bf16), processed in
PCH-tile slices with slice-local onehot staging (bufs rotate per slice):
  1. onehot [N, 128] -> SBUF bf16 slice tiles, layout [token-in-tile,
     (tile, g)] (HWDGE load + DVE cast).
  2. counts[g, t]: per 128-token tile, PE matmul lhsT=OH rhs=ones.
  3. prefix[g, t]: exclusive scan over tiles (DVE tensor_tensor_scan),
     PE-transpose to [t, g], add 128*g - 1.
  4. per GRP-tile PSUM group: flatten the GRP prefix rows to partition 0
     (tiny SBUF->SBUF DMA), broadcast-add via K=1 matmul (start), then GRP
     upper-triangular-matrix matmuls add the within-tile inclusive cumsum.
  5. flat_idx: DVE scalar_tensor_tensor (PSUM x onehot) with accum_out
     reducing over g (the one-hot selects its block's entry).
  6. per tile: one SWDGE indirect DMA gathers 128 x 1KB feature rows
     (hardware contract: EXACTLY one index per partition — [128, CT]
     multi-column offsets silently misexecute, [1,128]/[16,8] shapes wedge
     the device, dma_gather wedges on its library load in this env); per
     CT-tile chunk one HWDGE store DMA writes the rows to the output.

The gather stage is bound by SWDGE descriptor generation on the Q7 pair
(~1.6us per indirect DMA x 128 tiles x NB batches); multi-queue spreading
does not help because the per-DMA index allgather occupies all Q7 cores.
"""

from contextlib import ExitStack

import numpy as np

import concourse.bass as bass
import concourse.bacc as bacc
import concourse.mybir as mybir
import concourse.tile as tile
from concourse import bass_utils
from concourse.masks import make_identity, make_upper_triangular

P = 128  # partitions = tokens per tile = G (blocks) = NG_MAX
KERNEL_VERSION = 11  # bump on every meaningful kernel change (NEFF-cache buster)
N_CORES = 8
B_FULL = 16  # full batch dim
N_TOK = 16384  # tokens per batch
D_FEAT = 256  # feature dim
NB = B_FULL // N_CORES  # batches per core

FP32 = mybir.dt.float32
BF16 = mybir.dt.bfloat16
I32 = mybir.dt.int32


def build_nc(NB: int, N: int, D: int, CT: int = 8, GRP: int = 4, LC: int = 16,
             STG_BUFS: int = 8, IDX_BUFS: int = 6, REPS: int = 1,
             DYN_LOOP: int = 0, MODE: str = "full", SCRATCH: int = 65536,
             MG: int = 0, Q: int = 1, SALT: int = 0, PCH: int = 16,
             OH_BUFS: int = 3):
    """Build the per-core bass program.

    NB: batches per core; N: tokens per batch; D: feature dim.
    CT: tiles per store chunk; GRP: tiles per PSUM group; LC: tiles per
    onehot load chunk.
    """
    T = N // P  # token tiles per batch
    assert T * P == N
    GRP = min(GRP, T)
    CT = min(CT, T)
    LC = min(LC, T)
    assert T % GRP == 0 and T % CT == 0 and T % LC == 0
    add = mybir.AluOpType.add
    mult = mybir.AluOpType.mult
    bypass = mybir.AluOpType.bypass

    nc = bacc.Bacc("TRN2", target_bir_lowering=False, debug=False,
                   dynamic_dma_scratch_size=SCRATCH, num_swdge_queues=Q)

    def _q(inst, i):
        qi = i % Q
        if qi:
            inst.ins.queue = f"qPoolDynamic{qi}"
        return inst

    def _store_eng(i):
        if SALT and (i % 2):
            return nc.scalar
        return nc.sync

    feat = nc.dram_tensor("block_features", [NB * N, D], FP32, kind="ExternalInput")
    oh = nc.dram_tensor("block_onehot", [NB, N, P], FP32, kind="ExternalInput")
    out = nc.dram_tensor("out", [NB, N, D], FP32, kind="ExternalOutput")
    gidx = None
    if MODE == "gather_in":
        # debug: externally supplied flat indices, [p, t] layout per batch
        gidx = nc.dram_tensor("gidx", [NB, P, N // P], I32, kind="ExternalInput")
    # The PJRT NEFF cache keys on the HLO alone (the embedded bass program
    # does not enter the hash), so distinct kernel versions collide.  A dummy
    # input whose shape encodes a version nonce forces a distinct hash.
    import zlib as _zlib
    _nonce = (
        _zlib.crc32(
            f"v{KERNEL_VERSION}-{NB}-{N}-{D}-{CT}-{GRP}-{LC}-{STG_BUFS}-{REPS}-{DYN_LOOP}-{MODE}-{SCRATCH}-{MG}-{Q}-{SALT}-{PCH}-{OH_BUFS}".encode()
        )
        % 4093
        + 1
    )
    nc.dram_tensor("version_tag", [1, _nonce], FP32, kind="ExternalInput")

    with tile.TileContext(nc) as tc, ExitStack() as ctx:
        cpool = ctx.enter_context(tc.tile_pool(name="const", bufs=1))
        ohpool = ctx.enter_context(tc.tile_pool(name="ohp", bufs=OH_BUFS))
        ldpool = ctx.enter_context(tc.tile_pool(name="ld", bufs=2))
        wpool = ctx.enter_context(tc.tile_pool(name="work", bufs=2))
        ppool = ctx.enter_context(tc.tile_pool(name="psum", bufs=2, space="PSUM"))
        pspool = ctx.enter_context(tc.tile_pool(name="psumsm", bufs=2, space="PSUM"))
        spool = ctx.enter_context(tc.tile_pool(name="stage", bufs=STG_BUFS))
        fpool = ctx.enter_context(tc.tile_pool(name="flat", bufs=2))
        ipool = ctx.enter_context(tc.tile_pool(name="idx", bufs=IDX_BUFS))

        # --- constants ---
        triu = cpool.tile([P, P], BF16)  # triu[k, m] = 1 iff k <= m
        make_upper_triangular(nc, triu[:], val=1.0, diag=True)
        ident = cpool.tile([P, P], FP32)
        make_identity(nc, ident[:])
        ones_col = cpool.tile([P, 1], BF16)
        nc.gpsimd.memset(ones_col[:], 1.0)
        ones_row = cpool.tile([1, P], FP32)
        nc.gpsimd.memset(ones_row[:], 1.0)
        # gmat[t, g] = 128*g - 1 (same every row)
        gmat_i = cpool.tile([P, P], I32)
        nc.gpsimd.iota(gmat_i[:], pattern=[[P, P]], base=-1, channel_multiplier=0)
        gmat = cpool.tile([P, P], FP32)
        nc.vector.tensor_copy(gmat[:], gmat_i[:])

        import contextlib
        loop_cm = tc.For_i(0, DYN_LOOP, 1) if DYN_LOOP else contextlib.nullcontext()
        with loop_cm:
          for rep in range(REPS):
            for b in range(NB):
                oh_src = oh.ap()[b].rearrange("(t p) g -> p t g", p=P)
                out_dst = out.ap()[b].rearrange("(t p) d -> p t d", p=P)

                if MODE == "gather_in":
                    idxs = ldpool.tile([P, T], I32, tag="gidx")
                    nc.scalar.dma_start(out=idxs[:], in_=gidx.ap()[b])
                    for c in range(T // CT):
                        stg = spool.tile([P, CT * D], FP32, tag="stg")
                        if MG:
                            idx_c = ipool.tile([P, CT], I32, tag="idxc")
                            nc.vector.tensor_copy(
                                idx_c[:], idxs[:, c * CT : (c + 1) * CT]
                            )
                            nc.gpsimd.indirect_dma_start(
                                out=stg[:],
                                out_offset=None,
                                in_=feat.ap(),
                                in_offset=bass.IndirectOffsetOnAxis(
                                    ap=idx_c[:], axis=0
                                ),
                                element_offset=b * N * D,
                            )
                        else:
                            for i in range(CT):
                                t = c * CT + i
                                _q(nc.gpsimd.indirect_dma_start(
                                    out=stg[:, i * D : (i + 1) * D],
                                    out_offset=None,
                                    in_=feat.ap(),
                                    in_offset=bass.IndirectOffsetOnAxis(
                                        ap=idxs[:, t : t + 1], axis=0
                                    ),
                                    element_offset=b * N * D,
                                ), t)
                        _store_eng(c).dma_start(
                            out=out_dst[:, c * CT : (c + 1) * CT, :], in_=stg[:]
                        )
                    continue
                if MODE == "gather":
                    idx_mat = wpool.tile([P, P], I32, tag="idxmat")
                    nc.gpsimd.iota(idx_mat[:, :T], pattern=[[1, T]], base=0,
                                   channel_multiplier=T)
                    for c in range(T // CT):
                        stg = spool.tile([P, CT * D], FP32, tag="stg")
                        if MG:
                            nc.gpsimd.indirect_dma_start(
                                out=stg[:],
                                out_offset=None,
                                in_=feat.ap(),
                                in_offset=bass.IndirectOffsetOnAxis(
                                    ap=idx_mat[:, c * CT : (c + 1) * CT], axis=0
                                ),
                                element_offset=b * N * D,
                            )
                        else:
                            for i in range(CT):
                                t = c * CT + i
                                _q(nc.gpsimd.indirect_dma_start(
                                    out=stg[:, i * D : (i + 1) * D],
                                    out_offset=None,
                                    in_=feat.ap(),
                                    in_offset=bass.IndirectOffsetOnAxis(
                                        ap=idx_mat[:, t : t + 1], axis=0
                                    ),
                                    element_offset=b * N * D,
                                ), t)
                        _store_eng(c).dma_start(
                            out=out_dst[:, c * CT : (c + 1) * CT, :], in_=stg[:]
                        )
                    continue
                countsT_ps = pspool.tile([P, T], FP32, tag="counts")
                incl = wpool.tile([P, P], FP32, tag="incl")  # [g, t] inclusive
                pexT = wpool.tile([P, P], FP32, tag="pexT")  # [g, t] exclusive
                flat_mat = wpool.tile([P, P], FP32, tag="flatmat")  # [p', t]
                idx_mat = wpool.tile([P, P], I32, tag="idxmat")
                PCH_ = min(PCH, T)  # tiles per prefix/transpose slice
                assert PCH_ % GRP == 0 and PCH_ % CT == 0 and PCH_ % LC == 0
                oh_slices = {}
                for s in range(T // PCH_):
                    s0 = s * PCH_
                    oh_sb = ohpool.tile([P, PCH_ * P], BF16, tag="oh")
                    oh_slices[s] = oh_sb
                    # --- load + cast + counts + scan per LC chunk ---
                    for lc in range(s0 // LC, (s0 + PCH_) // LC):
                        lc0, lc1 = lc * LC, (lc + 1) * LC
                        ld = ldpool.tile([P, LC * P], FP32, tag="ld")
                        nc.scalar.dma_start(
                            out=ld[:], in_=oh_src[:, lc0:lc1, :]
                        )
                        nc.vector.tensor_copy(
                            oh_sb[:, (lc0 - s0) * P : (lc1 - s0) * P], ld[:]
                        )
                        for t in range(lc0, lc1):
                            nc.tensor.matmul(
                                out=countsT_ps[:, t : t + 1],
                                lhsT=oh_sb[:, (t - s0) * P : (t - s0 + 1) * P],
                                rhs=ones_col[:],
                                start=True,
                                stop=True,
                            )
                        nc.vector.tensor_tensor_scan(
                            out=incl[:, lc0:lc1],
                            data0=countsT_ps[:, lc0:lc1],
                            data1=gmat[:, 0:LC],
                            initial=(0.0 if lc == 0 else incl[:, lc0 - 1 : lc0]),
                            op0=add,
                            op1=bypass,
                        )
                        nc.vector.tensor_tensor(
                            out=pexT[:, lc0:lc1],
                            in0=incl[:, lc0:lc1],
                            in1=countsT_ps[:, lc0:lc1],
                            op=mybir.AluOpType.subtract,
                        )
                    # --- transpose slice into partition-0 tiles + gmat add ---
                    pref_ps = pspool.tile([PCH_, P], FP32, tag="preft")
                    nc.tensor.transpose(
                        out=pref_ps[:],
                        in_=pexT[:, s0 : s0 + PCH_],
                        identity=ident[:],
                    )
                    pref_adj = wpool.tile([PCH_, P], FP32, tag="prefadj")
                    nc.vector.tensor_tensor(
                        out=pref_adj[:],
                        in0=pref_ps[:],
                        in1=gmat[0:PCH_, :],
                        op=add,
                    )
                    # --- groups: prefix broadcast + within-tile cumsum + select ---
                    for grp in range(s0 // GRP, (s0 + PCH_) // GRP):
                        flat_row = fpool.tile([1, GRP * P], FP32, tag="flatrow")
                        g_in_s = grp - s0 // GRP
                        nc.scalar.dma_start(
                            out=flat_row[:],
                            in_=pref_adj[g_in_s * GRP : (g_in_s + 1) * GRP, :],
                        )
                        grp_ps = ppool.tile([P, GRP * P], FP32, tag="grp")
                        nc.tensor.matmul(
                            out=grp_ps[:],
                            lhsT=ones_row[:],
                            rhs=flat_row[:],
                            start=True,
                            stop=False,
                            skip_group_check=True,
                        )
                        for i in range(GRP):
                            t = grp * GRP + i
                            nc.tensor.matmul(
                                out=grp_ps[:, i * P : (i + 1) * P],
                                lhsT=triu[:],
                                rhs=oh_sb[:, (t - s0) * P : (t - s0 + 1) * P],
                                start=False,
                                stop=True,
                                skip_group_check=True,
                            )
                        scratch = wpool.tile([P, GRP * P], FP32, tag="scratch")
                        for i in range(GRP):
                            t = grp * GRP + i
                            nc.vector.scalar_tensor_tensor(
                                out=scratch[:, i * P : (i + 1) * P],
                                in0=grp_ps[:, i * P : (i + 1) * P],
                                scalar=1.0,
                                in1=oh_sb[:, (t - s0) * P : (t - s0 + 1) * P],
                                op0=mult,
                                op1=mult,
                                accum_out=flat_mat[:, t : t + 1],
                            )
                    if MODE == "index":
                        nc.sync.dma_start(
                            out=out_dst[:, s0 : s0 + 1, 0:1],
                            in_=flat_mat[:, s0 : s0 + 1],
                        )
                        continue
                    # --- gathers + stores for this slice ---
                    for c in range(s0 // CT, (s0 + PCH_) // CT):
                        stg = spool.tile([P, CT * D], FP32, tag="stg")
                        if MG:
                            idx_c = ipool.tile([P, CT], I32, tag="idxc")
                            nc.vector.tensor_copy(
                                idx_c[:], flat_mat[:, c * CT : (c + 1) * CT]
                            )
                            nc.gpsimd.indirect_dma_start(
                                out=stg[:],
                                out_offset=None,
                                in_=feat.ap(),
                                in_offset=bass.IndirectOffsetOnAxis(
                                    ap=idx_c[:], axis=0
                                ),
                                element_offset=b * N * D,
                            )
                        else:
                            nc.vector.tensor_copy(
                                idx_mat[:, c * CT : (c + 1) * CT],
                                flat_mat[:, c * CT : (c + 1) * CT],
                            )
                            for i in range(CT):
                                t = c * CT + i
                                _q(nc.gpsimd.indirect_dma_start(
                                    out=stg[:, i * D : (i + 1) * D],
                                    out_offset=None,
                                    in_=feat.ap(),
                                    in_offset=bass.IndirectOffsetOnAxis(
                                        ap=idx_mat[:, t : t + 1], axis=0
                                    ),
                                    element_offset=b * N * D,
                                ), t)
                        _store_eng(c).dma_start(
                            out=out_dst[:, c * CT : (c + 1) * CT, :], in_=stg[:]
                        )

    nc.compile()
    return nc


_NC_CACHE = {}


def _get_nc():
    key = (NB, N_TOK, D_FEAT)
    if key not in _NC_CACHE:
        _NC_CACHE[key] = build_nc(*key)
    return _NC_CACHE[key]


def _tag_shape(nc):
    for alloc in nc.m.functions[0].allocations:
        if isinstance(alloc, mybir.MemoryLocationSet) and alloc.kind == "ExternalInput":
            if alloc.memorylocations[0].name == "version_tag":
                return tuple(alloc.tensor_shape)
    return None


def fix_maps(nc, in_maps):
    """Adjust the version_tag entry of in_maps to match nc's nonce shape."""
    shape = _tag_shape(nc)
    maps = [dict(m) for m in in_maps]
    for m in maps:
        m.pop("version_tag", None)
        if shape is not None:
            m["version_tag"] = np.zeros(shape, np.float32)
    return maps


def make_in_maps(block_features: np.ndarray, block_onehot: np.ndarray):
    """Shard full inputs batch-wise into 8 per-core input maps."""
    feat = np.ascontiguousarray(block_features, dtype=np.float32).reshape(
        B_FULL, N_TOK, D_FEAT
    )
    oh = np.ascontiguousarray(block_onehot, dtype=np.float32)
    nc = _get_nc()
    tag_shape = _tag_shape(nc)
    in_maps = []
    for c in range(N_CORES):
        lo, hi = c * NB, (c + 1) * NB
        m = {
            "block_features": feat[lo:hi].reshape(NB * N_TOK, D_FEAT),
            "block_onehot": oh[lo:hi],
        }
        if tag_shape is not None:
            m["version_tag"] = np.zeros(tag_shape, np.float32)
        in_maps.append(m)
    return in_maps


def run_spmd(in_maps, trace: bool = False):
    """Compile (cached) + run the SPMD program on cores 0-7."""
    nc = _get_nc()
    return bass_utils.run_bass_kernel_spmd(
        nc, in_maps, core_ids=list(range(N_CORES)), trace=trace
    )


def kernel(**inputs) -> np.ndarray:
    block_features = inputs["block_features"]
    block_onehot = inputs["block_onehot"]
    in_maps = make_in_maps(block_features, block_onehot)
    res = run_spmd(in_maps, trace=False)
    out = np.concatenate([r["out"] for r in res.results], axis=0)
    return out.reshape(B_FULL, N_TOK, D_FEAT)

